# revision 72
# baseline (speedup 1.0000x reference)
"""Segment-mean (MeanToERA5) Trainium2 kernel.

Computes per-cluster means of a [32, 8, 512, 512] fp32 tensor over the
flattened 512x512 spatial axis, for 4096 clusters given by `mapping`
([262144] int), matching jax.ops.segment_sum(flat.T, mapping)/counts.

Strategy (8 NeuronCores, SPMD):
  - Host: stable-argsort `mapping`; group the 4096 clusters into groups of
    G=32 consecutive clusters; lay out the data cluster-sorted and
    transposed as rows of [256 batch], padded per-group to a uniform
    row count 128*cpg so the program structure is identical on every
    core. Each core owns 512 clusters = 16 groups. Inputs are packed
    partition-major on the host so all DMAs are fully contiguous.
  - Precision: the harness gate is rel_err < 2e-2; data is cast on the
    host to bf16 (measured end-to-end rel err 1.7e-3) or quantized to
    int8 with per-row scales folded into the one-hot weights (measured
    6.5e-3), halving/quartering HBM traffic vs fp32.
  - Device: build per-chunk [128, 32] one-hot weights on DVE from a
    compact column-id vector; per 128-row chunk one bf16 matmul:
    stationary = one-hot, moving = data chunk [128, 256]. PSUM
    accumulates [512 clusters, 256 batch] c-major in 4 [128, 256]
    tiles; multiply by 1/count on the PSUM->SBUF copy and DMA out.
  - Host: assemble [4096, 256], transpose to [256, 4096] (the unshard).
"""

import sys
import time

if "/opt/trn_rl_repo" not in sys.path:
    sys.path.insert(0, "/opt/trn_rl_repo")

import numpy as np
import jax

# Persistent JAX compilation cache: the NEFF compile is reused across
# processes for identical programs.
try:
    if jax.config.jax_compilation_cache_dir is None:
        jax.config.update("jax_compilation_cache_dir", "/tmp/jax_neff_cache")
    jax.config.update("jax_persistent_cache_min_entry_size_bytes", -1)
    jax.config.update("jax_persistent_cache_min_compile_time_secs", 0.1)
except Exception:
    pass

import ml_dtypes
import concourse.bacc as bacc
import concourse.tile as tile
from concourse import mybir
from concourse.bass_utils import run_bass_kernel_spmd

N_CLUSTERS = 4096
N = 512 * 512
B = 256
NCORES = 8
G = 32                      # clusters per group (= one-hot width)
GROUPS_PER_CORE = (N_CLUSTERS // NCORES) // G   # 16
CLUSTERS_PER_CORE = N_CLUSTERS // NCORES        # 512
NQ = CLUSTERS_PER_CORE // 128                   # psum tiles (4)
GPD = 4                     # groups per DMA (DMA transfer = GPD MiB bf16)

QUANT = "bf16"              # "bf16" | "int8"
SHIP_OH = False             # host pre-builds the one-hot weights (no DVE build)
# Pair consecutive chunks with identical partition->cluster patterns so one
# N=512 matmul covers two chunks (half the LDWEIGHTS/matmul instructions,
# denser PE streams, half the one-hot build). bf16 only. Measured: the
# +12.5% padding bytes cost more than the PE restructure saves (the kernel
# is DMA-byte-bound) — keep off.
PAIRED = False
# Fusing cid+iota+recip into one aux DMA measured ~3us SLOWER (65.7/66.3 vs
# 62.4-63.9 unfused across 5 runs) — keep off.
FUSE_AUX = False

_program_cache = {}


def _build_program(cpg: int, loop: int = 1, quant: str | None = None):
    """Build the SPMD bass program for `cpg` 128-row chunks per group.

    loop > 1 repeats the whole pipeline on-device (for benchmarking: one
    dispatch, `loop` executions)."""
    if quant is None:
        quant = QUANT
    key = (cpg, loop, quant)
    if key in _program_cache:
        return _program_cache[key]

    paired = PAIRED and quant == "bf16"
    nchunks = GROUPS_PER_CORE * cpg    # chunks per core
    # chunks per one-hot pattern (2 when paired) and patterns per group
    kc = 2 if paired else 1
    cpp = cpg // kc                    # patterns per group
    npat = GROUPS_PER_CORE * cpp
    gpq = 128 // G                     # groups per psum tile (4)
    ndma = GROUPS_PER_CORE // GPD      # x transfers per iteration
    bf16 = mybir.dt.bfloat16
    xdt = mybir.dt.int8 if quant == "int8" else bf16

    nc = bacc.Bacc("TRN2", target_bir_lowering=False, debug=False,
                   num_devices=NCORES)
    # x packed as [ndma, 128 partitions, GPD*cpg*B] (host pre-permuted)
    x = nc.dram_tensor("x", [ndma, 128, GPD * cpg * B], xdt,
                       kind="ExternalInput")
    if SHIP_OH:
        # host-prebuilt one-hot weights, packed per DMA block
        oh = nc.dram_tensor("oh", [ndma, 128, GPD * cpg * G], bf16,
                            kind="ExternalInput")
    use_aux = FUSE_AUX and quant == "bf16" and not SHIP_OH and not paired
    if use_aux:
        # cid + iota + recip(bf16) fused into one small input -> one DMA
        aux = nc.dram_tensor("aux", [128, npat + G + 2 * NQ], bf16,
                             kind="ExternalInput")
    elif not SHIP_OH:
        # per-pattern one-hot column id, packed [128, npat]
        cid = nc.dram_tensor("cid", [128, npat], bf16,
                             kind="ExternalInput")
        iota = nc.dram_tensor("iota", [128, G], bf16, kind="ExternalInput")
    if quant == "int8":
        # per-(group, partition) dequant scales
        sg = nc.dram_tensor("sg", [128, GROUPS_PER_CORE], mybir.dt.float32,
                            kind="ExternalInput")
    if not use_aux:
        # per-psum-tile per-partition 1/count
        recip = nc.dram_tensor("recip", [128, NQ], mybir.dt.float32,
                               kind="ExternalInput")
    # output partition-major: [p, q*B+b] holds cluster q*128+p, batch b
    # (one fully contiguous DMA; host untangles)
    out = nc.dram_tensor("out", [128, NQ * B], mybir.dt.float32,
                         kind="ExternalOutput")

    xv, outv = x.ap(), out.ap()

    import contextlib
    with tile.TileContext(nc) as tc:
        with contextlib.ExitStack() as stk:
            xp = stk.enter_context(tc.tile_pool(name="xp", bufs=1))
            if quant == "int8":
                stp = stk.enter_context(tc.tile_pool(name="sp", bufs=1))
            ohp = stk.enter_context(tc.tile_pool(name="ohp", bufs=2))
            ps = stk.enter_context(
                tc.tile_pool(name="ps", bufs=2, space="PSUM"))
            resp = stk.enter_context(tc.tile_pool(name="res", bufs=2))
            def body(_i=None):
                # int8: keep the ACT queue free for dequant ops — all small
                # DMAs go via SWDGE (gpsimd), all x streams via sync
                small_eng = nc.gpsimd if quant == "int8" else nc.scalar
                if use_aux:
                    auxt = ohp.tile([128, npat + G + 2 * NQ], bf16,
                                    name="auxt", tag="auxt")
                    nc.scalar.dma_start(auxt[:], aux.ap())
                    cidt = auxt[:, :npat]
                    iot = auxt[:, npat:npat + G]
                    # recip rides along as fp32 bit-split into bf16 pairs
                    rect = auxt[:, npat + G:npat + G + 2 * NQ].bitcast(
                        mybir.dt.float32)
                else:
                    rect = ohp.tile([128, NQ], mybir.dt.float32,
                                    name="rect", tag="rect")
                    small_eng.dma_start(rect[:], recip.ap())
                if quant == "int8":
                    sgt = ohp.tile([128, GROUPS_PER_CORE],
                                   mybir.dt.float32, name="sgt", tag="sgt")
                    small_eng.dma_start(sgt[:], sg.ap())
                ohx = ohp.tile([128, npat, G], bf16, name="ohx",
                               tag="ohx")
                if SHIP_OH:
                    ohv = oh.ap()
                    for d in range(ndma):
                        s = slice(d * GPD * cpp, (d + 1) * GPD * cpp)
                        eng = nc.scalar if d % 2 == 0 else nc.sync
                        eng.dma_start(ohx[:, s, :], ohv[d])
                else:
                    if not use_aux:
                        cidt = ohp.tile([128, npat], bf16, name="cidt",
                                        tag="cidt")
                        small_eng.dma_start(cidt[:], cid.ap())
                        iot = ohp.tile([128, G], bf16, name="iot",
                                       tag="iot")
                        small_eng.dma_start(iot[:], iota.ap())
                    for g in range(GROUPS_PER_CORE):
                        s = slice(g * cpp, (g + 1) * cpp)
                        nc.vector.tensor_tensor(
                            out=ohx[:, s, :],
                            in0=cidt[:, s].unsqueeze(2)
                                .broadcast_to([128, cpp, G]),
                            in1=iot[:].unsqueeze(1)
                                .broadcast_to([128, cpp, G]),
                            op=mybir.AluOpType.is_equal,
                        )
                psum = [
                    ps.tile([128, kc * B], mybir.dt.float32,
                            name=f"psum{q}", tag=f"psum{q}")
                    for q in range(NQ)
                ]
                for d in range(ndma):
                    # alternate the two HWDGE rings (SP / ACT) so the
                    # per-dma fixed costs and streams run in parallel;
                    # int8 keeps ACT's queue clear -> all x on sync
                    eng = nc.sync if (quant == "int8" or d % 2 == 0) \
                        else nc.scalar
                    xt = xp.tile([128, GPD * cpg * B], xdt, tag=f"x{d}")
                    eng.dma_start(xt[:], xv[d])
                    if quant == "int8":
                        # dequantize each group slice int8 -> bf16 on
                        # DVE/ACT, scaling by the per-(group, partition)
                        # scalar during the copy
                        stag = []
                        for gg in range(GPD):
                            u = d * GPD + gg
                            st = stp.tile([128, cpg * B], bf16,
                                          tag=f"s{u % 12}")
                            sl = xt[:, gg * cpg * B:(gg + 1) * cpg * B]
                            sc = sgt[:, u:u + 1]
                            # DVE also builds the one-hot, so it gets 6 of
                            # the 16 dequant slices and ACT the other 10
                            if u % 3 == 0:
                                nc.vector.tensor_scalar_mul(st[:], sl, sc)
                            else:
                                nc.scalar.mul(st[:], sl, sc)
                            stag.append(st)
                    # issue the 4 groups of this block column-interleaved:
                    # consecutive matmuls hit different 32-col strips of the
                    # PE array (tile_position), so they run concurrently.
                    for t in range(cpp):
                        for gg in range(GPD):
                            g = d * GPD + gg
                            q, gq = divmod(g, gpq)
                            po = gq * G    # partition offset in psum tile
                            j = g * cpp + t
                            if quant == "int8":
                                rhs = stag[gg][:, t * B:(t + 1) * B]
                            else:
                                c0 = (gg * cpg + kc * t) * B
                                rhs = xt[:, c0:c0 + kc * B]
                            nc.tensor.matmul(
                                out=psum[q][po:po + G, :],
                                lhsT=ohx[:, j, :],
                                rhs=rhs,
                                start=(t == 0),
                                stop=(t == cpp - 1),
                                tile_position=(0, po),
                            )
                if use_aux:
                    # one wide result tile, scaled per psum tile, shipped
                    # with a single contiguous DMA
                    res_all = resp.tile([128, NQ * B], mybir.dt.float32,
                                        name="res_all", tag="res")
                    for q in range(NQ):
                        nc.vector.tensor_scalar_mul(
                            res_all[:, q * B:(q + 1) * B], psum[q][:],
                            rect[:, q:q + 1])
                    nc.sync.dma_start(outv[:], res_all[:])
                    return
                for q in range(NQ):
                    res = resp.tile([128, B], mybir.dt.float32,
                                    name=f"res{q}", tag="res")
                    if paired:
                        # the two pair-halves land side by side in PSUM;
                        # only one PSUM operand is allowed per op, so scale
                        # each half separately (DVE + ACT) and add in SBUF
                        tmp = resp.tile([128, B], mybir.dt.float32,
                                        name=f"tmp{q}", tag="tmp")
                        nc.vector.tensor_scalar_mul(
                            res[:], psum[q][:, :B], rect[:, q:q + 1])
                        nc.scalar.mul(tmp[:], psum[q][:, B:2 * B],
                                      rect[:, q:q + 1])
                        nc.vector.tensor_tensor(
                            out=res[:], in0=res[:], in1=tmp[:],
                            op=mybir.AluOpType.add,
                        )
                    else:
                        nc.vector.tensor_tensor(
                            out=res[:], in0=psum[q][:],
                            in1=rect[:, q:q + 1].broadcast_to([128, B]),
                            op=mybir.AluOpType.mult,
                        )
                    if quant == "int8":
                        nc.gpsimd.dma_start(
                            outv[:, q * B:(q + 1) * B], res[:])
                    else:
                        eng = nc.sync if q % 2 == 0 else nc.scalar
                        eng.dma_start(outv[:, q * B:(q + 1) * B],
                                      res[:])

            if loop == 1:
                body()
            else:
                with tc.For_i(0, loop, 1, staggered_reset=True) as i:
                    body(i)

    nc.compile()
    _program_cache[key] = nc
    return nc


def _solve_bins(counts: np.ndarray):
    """Partition the 4096 clusters into 128 bins of exactly 32 clusters,
    equalizing bin row-sums (ideally all == 2048 -> zero padding). Returns
    (bin_of, slot_of) int arrays."""
    n_bins = N_CLUSTERS // G
    target = int(counts.sum()) // n_bins
    rng = np.random.default_rng(0)
    orderd = np.argsort(-counts)
    bins = [[] for _ in range(n_bins)]
    sums = np.zeros(n_bins, dtype=np.int64)
    nitems = np.zeros(n_bins, dtype=np.int64)
    for c in orderd:
        cand = np.where(nitems < G)[0]
        b = int(cand[np.argmin(sums[cand])])
        bins[b].append(int(c))
        sums[b] += counts[c]
        nitems[b] += 1
    for _ in range(300000):
        dev = sums - target
        over = np.where(dev > 0)[0]
        under = np.where(dev < 0)[0]
        if len(over) == 0 or len(under) == 0:
            break
        A = int(rng.choice(over))
        Bb = int(rng.choice(under))
        ca, cb = bins[A], bins[Bb]
        diff = counts[ca][:, None] - counts[cb][None, :]
        tot = np.abs(dev[A] - diff) + np.abs(dev[Bb] + diff)
        i, j = np.unravel_index(int(np.argmin(tot)), tot.shape)
        if tot[i, j] < abs(dev[A]) + abs(dev[Bb]):
            a, b2 = ca[i], cb[j]
            ca.remove(a), cb.remove(b2)
            ca.append(b2), cb.append(a)
            d = counts[a] - counts[b2]
            sums[A] -= d
            sums[Bb] += d
    bin_of = np.zeros(N_CLUSTERS, dtype=np.int64)
    slot_of = np.zeros(N_CLUSTERS, dtype=np.int64)
    for b, cl in enumerate(bins):
        bin_of[cl] = b
        slot_of[cl] = np.arange(len(cl))
    return bin_of, slot_of, int(sums.max())


def _prepare(output: np.ndarray, mapping: np.ndarray):
    """Host prep: returns (nc, in_maps, cpg, unperm)."""
    t0 = time.time()
    assert output.shape == (32, 8, 512, 512) and output.dtype == np.float32
    mapping = np.asarray(mapping).astype(np.int64).ravel()
    assert mapping.shape == (N,)

    data2d = output.reshape(B, N)
    counts = np.bincount(mapping, minlength=N_CLUSTERS).astype(np.int64)
    recip = (1.0 / np.maximum(counts, 1)).astype(np.float32)

    order = np.argsort(mapping, kind="stable")
    cum = np.zeros(N_CLUSTERS + 1, dtype=np.int64)
    np.cumsum(counts, out=cum[1:])

    n_groups = N_CLUSTERS // G
    paired = PAIRED and QUANT == "bf16"
    # In paired mode a cluster's rows occupy whole chunk-pairs, so its
    # effective row count is rounded up to even.
    counts_eff = (counts + 1) // 2 * 2 if paired else counts
    # Bin-pack clusters into groups to minimize padding; fall back to
    # consecutive grouping if the packer leaves an oversized bin.
    bin_of, slot_of, maxsum = _solve_bins(counts_eff)
    naive_max = int(np.add.reduceat(counts_eff,
                                    np.arange(0, N_CLUSTERS, G)).max())
    if maxsum > naive_max:
        bin_of = np.arange(N_CLUSTERS) // G
        slot_of = np.arange(N_CLUSTERS) % G
        maxsum = naive_max
    if paired:
        cpg = max(2, 2 * int(np.ceil(maxsum / 256)))
    else:
        cpg = max(1, int(np.ceil(maxsum / 128)))
    L = 128 * cpg

    # clusters in destination order (bin-major, slot order)
    dest_order = np.lexsort((slot_of, bin_of))
    glen = np.zeros(n_groups, dtype=np.int64)
    np.add.at(glen, bin_of, counts_eff)
    rows_sorted = np.concatenate(
        [order[cum[c]:cum[c + 1]] for c in dest_order])
    gstart = np.zeros(n_groups + 1, dtype=np.int64)
    np.cumsum(glen, out=gstart[1:])

    if paired:
        # Pair-based placement: pair j of a cluster occupies positions
        # (chunk 2T, p) and (chunk 2T+1, p) of its group, so both chunks
        # of a pair share one partition->slot one-hot pattern.
        counts_dest = counts[dest_order]               # true counts
        pairs_of = (counts_dest + 1) // 2
        seg = np.zeros(len(dest_order) + 1, dtype=np.int64)
        np.cumsum(counts_dest, out=seg[1:])            # rows_sorted segs
        tot_pairs = int(pairs_of.sum())
        pstart = np.zeros(len(dest_order) + 1, dtype=np.int64)
        np.cumsum(pairs_of, out=pstart[1:])
        within = np.arange(tot_pairs) - np.repeat(pstart[:-1], pairs_of)
        r0 = np.repeat(seg[:-1], pairs_of) + 2 * within
        cnt_rep = np.repeat(counts_dest, pairs_of)
        has_r1 = (2 * within + 1) < cnt_rep
        # group and per-group pair index (dest order is bin-major)
        dpos = np.repeat(np.arange(len(dest_order)), pairs_of)
        pair_group = dpos // G
        pair_slot = (dpos % G).astype(np.int64)
        gp_pairs = np.add.reduceat(pairs_of, np.arange(0, N_CLUSTERS, G))
        gp_start = np.zeros(n_groups + 1, dtype=np.int64)
        np.cumsum(gp_pairs, out=gp_start[1:])
        ppi = np.arange(tot_pairs) - np.repeat(gp_start[:-1], gp_pairs)
        T = ppi // 128
        p = ppi % 128
        assert int(T.max()) < cpg // 2
        pad_rows = np.full(n_groups * L, -1, dtype=np.int64)
        base = pair_group * L + p
        pad_rows[base + (2 * T) * 128] = rows_sorted[r0]
        pad_rows[(base + (2 * T + 1) * 128)[has_r1]] = \
            rows_sorted[(r0 + 1)[has_r1]]
        vmask = pad_rows >= 0
        # per-pattern column id table [n_groups * (L//2)]
        cid_pat = np.zeros(n_groups * (L // 2), dtype=ml_dtypes.bfloat16)
        cid_pat[pair_group * (L // 2) + T * 128 + p] = \
            pair_slot.astype(ml_dtypes.bfloat16)
    else:
        # Padded row-id table [n_groups, L]; -1 = padding.
        pad_rows = np.full((n_groups, L), -1, dtype=np.int64)
        col = np.arange(L)
        valid = col[None, :] < glen[:, None]
        flat_src = np.zeros((n_groups, L), dtype=np.int64)
        flat_src[valid] = rows_sorted[
            (gstart[:-1][:, None]
             + np.minimum(col[None, :], glen[:, None] - 1))[valid]
        ]
        pad_rows[valid] = flat_src[valid]
        pad_rows = pad_rows.reshape(-1)        # [n_groups * L]
        vmask = pad_rows >= 0

    # Gather data rows (transposed): x_rows[r] = data2d[:, pad_rows[r]]
    dataT = np.ascontiguousarray(data2d.T)          # [N, B]
    if QUANT == "int8":
        # Magnitude-sorted placement: within each group, rank rows by
        # max|row| and place rank r at (chunk r%cpg, partition r//cpg) so
        # each partition holds rows of similar magnitude. Quantize with a
        # per-(group, partition) scale; the device applies it as a per-
        # partition scalar during the int8->bf16 dequant copy.
        pr2 = pad_rows.reshape(n_groups, L)
        vm2 = vmask.reshape(n_groups, L)
        rmax = np.full((n_groups, L), -1.0, dtype=np.float32)
        rmax[vm2] = np.abs(dataT[pr2[vm2]]).max(axis=1)
        rk = np.argsort(rmax, axis=1, kind="stable")   # padding first
        rows_rk = np.take_along_axis(pr2, rk, axis=1)  # rank-ordered rows
        rmax_rk = np.take_along_axis(rmax, rk, axis=1)
        # scale per (group, partition): partition p holds ranks
        # [p*cpg, (p+1)*cpg)
        sgmat = (np.maximum(rmax_rk.reshape(n_groups, 128, cpg).max(axis=2),
                            1e-30) / 127.0).astype(np.float32)
        # rank r -> position (chunk r%cpg)*128 + (partition r//cpg)
        pos = (np.arange(L) % cpg) * 128 + (np.arange(L) // cpg)
        pr_new = np.empty_like(pr2)
        np.put_along_axis(pr_new, np.broadcast_to(pos, (n_groups, L)),
                          rows_rk, axis=1)
        pad_rows = pr_new.reshape(-1)
        vmask = pad_rows >= 0
        # per-position scale = scale of its partition
        s_pos = sgmat[:, np.arange(L) % 128].reshape(-1)   # [n_groups*L]
        x_rows = np.zeros((n_groups * L, B), dtype=np.int8)
        x_rows[vmask] = np.clip(
            np.round(dataT[pad_rows[vmask]] / s_pos[vmask][:, None]),
            -127, 127).astype(np.int8)
        # pack scales per core: [NCORES, 128, GROUPS_PER_CORE]
        sg_all = np.ascontiguousarray(
            sgmat.reshape(NCORES, GROUPS_PER_CORE, 128).transpose(0, 2, 1))
    else:
        x_rows = np.zeros((n_groups * L, B), dtype=ml_dtypes.bfloat16)
        x_rows[vmask] = dataT[pad_rows[vmask]].astype(ml_dtypes.bfloat16)
    # pack partition-major per DMA block: [dma, t, p, b] -> [dma, p, t*B+b]
    n_dma = n_groups // GPD
    x_all = np.ascontiguousarray(
        x_rows.reshape(n_dma, GPD * cpg, 128, B).transpose(0, 2, 1, 3)
    ).reshape(n_dma, 128, GPD * cpg * B)

    # Compact one-hot: per-row within-group column id (bf16).
    if paired:
        cid_all = cid_pat          # one column id per chunk-pair pattern
    else:
        cid_all = np.zeros(n_groups * L, dtype=ml_dtypes.bfloat16)
        clus = mapping[pad_rows[vmask]]
        cid_all[vmask] = slot_of[clus].astype(ml_dtypes.bfloat16)
    if SHIP_OH:
        # host-prebuilt one-hot [rows, G]
        clus = mapping[pad_rows[vmask]]
        w_rows = np.zeros(n_groups * L, dtype=np.float32)
        w_rows[vmask] = 1.0
        slot_rows = np.zeros(n_groups * L, dtype=np.int16)
        slot_rows[vmask] = slot_of[clus]
        oh_rows = (slot_rows[:, None] == np.arange(G, dtype=np.int16)[None]
                   ).astype(np.float32) * w_rows[:, None]
        # pack like x: [dma, chunk, p, G] -> [dma, p, chunk*G]
        oh_all = np.ascontiguousarray(
            oh_rows.reshape(n_dma, GPD * cpg, 128, G).transpose(0, 2, 1, 3)
        ).reshape(n_dma, 128, GPD * cpg * G).astype(ml_dtypes.bfloat16)
    # where cluster c ended up in the concatenated [4096, B] device output
    unperm = bin_of * G + slot_of
    # per-core per-psum-tile per-partition reciprocal counts
    counts_dest = counts[dest_order]               # [4096] device order
    recip_dev = (1.0 / np.maximum(counts_dest, 1)).astype(np.float32)
    recip_all = recip_dev.reshape(NCORES, NQ, 128).transpose(0, 2, 1)
    recip_all = np.ascontiguousarray(recip_all)    # [NCORES, 128, NQ]
    # pack [rows] -> [core][p][chunk]
    npat_core = GROUPS_PER_CORE * (cpg // 2 if paired else cpg)

    def pack(a):
        return np.ascontiguousarray(
            a.reshape(NCORES, npat_core, 128).transpose(0, 2, 1))

    cid_all = pack(cid_all)
    iota_np = np.broadcast_to(
        np.arange(G, dtype=ml_dtypes.bfloat16), (128, G)).copy()

    t1 = time.time()
    nc = _build_program(cpg)

    ndma_core = GROUPS_PER_CORE // GPD
    use_aux = FUSE_AUX and QUANT == "bf16" and not SHIP_OH and not paired
    in_maps = []
    for k in range(NCORES):
        m = {"x": x_all[k * ndma_core:(k + 1) * ndma_core]}
        if use_aux:
            m["aux"] = np.ascontiguousarray(np.concatenate([
                np.asarray(cid_all[k]),
                iota_np,
                np.ascontiguousarray(recip_all[k])
                .view(ml_dtypes.bfloat16),
            ], axis=1))
        else:
            m["recip"] = recip_all[k]
            if SHIP_OH:
                m["oh"] = oh_all[k * ndma_core:(k + 1) * ndma_core]
            else:
                m["cid"] = cid_all[k]
                m["iota"] = iota_np
        if QUANT == "int8":
            m["sg"] = sg_all[k]
        in_maps.append(m)
    print(f"[kernel] host prep {t1 - t0:.2f}s  build+compile "
          f"{time.time() - t1:.2f}s  (cpg={cpg}, quant={QUANT})",
          file=sys.stderr, flush=True)
    return nc, in_maps, cpg, unperm


def kernel(output: np.ndarray, mapping: np.ndarray) -> np.ndarray:
    nc, in_maps, _, unperm = _prepare(output, mapping)
    t2 = time.time()
    res = run_bass_kernel_spmd(nc, in_maps, list(range(NCORES)))
    t3 = time.time()
    # device out is partition-major [128, NQ*B]: [p, q*B+b] holds cluster
    # q*128+p -> untangle to [512, B] per core
    full = np.concatenate([
        res.results[k]["out"].reshape(128, NQ, B)
        .transpose(1, 0, 2).reshape(CLUSTERS_PER_CORE, B)
        for k in range(NCORES)
    ], axis=0)                                      # [4096, 256] device order
    full = full[unperm]                             # -> cluster order
    out = np.ascontiguousarray(full.T).reshape(32, 8, N_CLUSTERS)
    print(f"[kernel] run {t3 - t2:.2f}s", file=sys.stderr, flush=True)
    return out


# revision 73
# speedup vs baseline: 1.1713x; 1.1713x over previous
"""Segment-mean (MeanToERA5) Trainium2 kernel.

Computes per-cluster means of a [32, 8, 512, 512] fp32 tensor over the
flattened 512x512 spatial axis, for 4096 clusters given by `mapping`
([262144] int), matching jax.ops.segment_sum(flat.T, mapping)/counts.

Strategy (8 NeuronCores, SPMD):
  - Host: stable-argsort `mapping`; group the 4096 clusters into groups of
    G=32 consecutive clusters; lay out the data cluster-sorted and
    transposed as rows of [256 batch], padded per-group to a uniform
    row count 128*cpg so the program structure is identical on every
    core. Each core owns 512 clusters = 16 groups. Inputs are packed
    partition-major on the host so all DMAs are fully contiguous.
  - Precision: the harness gate is rel_err < 2e-2; data is cast on the
    host to bf16 (measured end-to-end rel err 1.7e-3) or quantized to
    int8 with per-row scales folded into the one-hot weights (measured
    6.5e-3), halving/quartering HBM traffic vs fp32.
  - Device: build per-chunk [128, 32] one-hot weights on DVE from a
    compact column-id vector; per 128-row chunk one bf16 matmul:
    stationary = one-hot, moving = data chunk [128, 256]. PSUM
    accumulates [512 clusters, 256 batch] c-major in 4 [128, 256]
    tiles; multiply by 1/count on the PSUM->SBUF copy and DMA out.
  - Host: assemble [4096, 256], transpose to [256, 4096] (the unshard).
"""

import sys
import time

if "/opt/trn_rl_repo" not in sys.path:
    sys.path.insert(0, "/opt/trn_rl_repo")

import numpy as np
import jax

# Persistent JAX compilation cache: the NEFF compile is reused across
# processes for identical programs.
try:
    if jax.config.jax_compilation_cache_dir is None:
        jax.config.update("jax_compilation_cache_dir", "/tmp/jax_neff_cache")
    jax.config.update("jax_persistent_cache_min_entry_size_bytes", -1)
    jax.config.update("jax_persistent_cache_min_compile_time_secs", 0.1)
except Exception:
    pass

import ml_dtypes
import concourse.bacc as bacc
import concourse.tile as tile
from concourse import mybir
from concourse.bass_utils import run_bass_kernel_spmd

N_CLUSTERS = 4096
N = 512 * 512
B = 256
NCORES = 8
G = 32                      # clusters per group (= one-hot width)
GROUPS_PER_CORE = (N_CLUSTERS // NCORES) // G   # 16
CLUSTERS_PER_CORE = N_CLUSTERS // NCORES        # 512
NQ = CLUSTERS_PER_CORE // 128                   # psum tiles (4)
GPD = 4                     # groups per DMA (DMA transfer = GPD MiB bf16)

QUANT = "bf16"              # "bf16" | "int8"
SHIP_OH = False             # host pre-builds the one-hot weights (no DVE build)
# Pair consecutive chunks with identical partition->cluster patterns so one
# N=512 matmul covers two chunks (half the LDWEIGHTS/matmul instructions,
# denser PE streams, half the one-hot build). bf16 only. Measured: the
# +12.5% padding bytes cost more than the PE restructure saves (the kernel
# is DMA-byte-bound) — keep off.
PAIRED = False
# Fusing cid+iota+recip into one aux DMA measured ~3us SLOWER (65.7/66.3 vs
# 62.4-63.9 unfused across 5 runs) — keep off.
FUSE_AUX = False

_program_cache = {}


def _build_program(cpg: int, loop: int = 1, quant: str | None = None):
    """Build the SPMD bass program for `cpg` 128-row chunks per group.

    loop > 1 repeats the whole pipeline on-device (for benchmarking: one
    dispatch, `loop` executions)."""
    if quant is None:
        quant = QUANT
    key = (cpg, loop, quant)
    if key in _program_cache:
        return _program_cache[key]

    paired = PAIRED and quant == "bf16"
    nchunks = GROUPS_PER_CORE * cpg    # chunks per core
    # chunks per one-hot pattern (2 when paired) and patterns per group
    kc = 2 if paired else 1
    cpp = cpg // kc                    # patterns per group
    npat = GROUPS_PER_CORE * cpp
    gpq = 128 // G                     # groups per psum tile (4)
    ndma = GROUPS_PER_CORE // GPD      # x transfers per iteration
    bf16 = mybir.dt.bfloat16
    xdt = mybir.dt.int8 if quant == "int8" else bf16

    nc = bacc.Bacc("TRN2", target_bir_lowering=False, debug=False,
                   num_devices=NCORES)
    # x packed as [ndma, 128 partitions, GPD*cpg*B] (host pre-permuted)
    x = nc.dram_tensor("x", [ndma, 128, GPD * cpg * B], xdt,
                       kind="ExternalInput")
    if SHIP_OH:
        # host-prebuilt one-hot weights, packed per DMA block
        oh = nc.dram_tensor("oh", [ndma, 128, GPD * cpg * G], bf16,
                            kind="ExternalInput")
    use_aux = FUSE_AUX and quant == "bf16" and not SHIP_OH and not paired
    if use_aux:
        # cid + iota + recip(bf16) fused into one small input -> one DMA
        aux = nc.dram_tensor("aux", [128, npat + G + 2 * NQ], bf16,
                             kind="ExternalInput")
    elif not SHIP_OH:
        # per-pattern one-hot column id, packed [128, npat]
        cid = nc.dram_tensor("cid", [128, npat], bf16,
                             kind="ExternalInput")
        iota = nc.dram_tensor("iota", [128, G], bf16, kind="ExternalInput")
    if quant == "int8":
        # per-(group, partition) dequant scales
        sg = nc.dram_tensor("sg", [128, GROUPS_PER_CORE], mybir.dt.float32,
                            kind="ExternalInput")
    if not use_aux:
        # per-psum-tile per-partition 1/count
        recip = nc.dram_tensor("recip", [128, NQ], mybir.dt.float32,
                               kind="ExternalInput")
    if use_aux:
        # output partition-major: [p, q*B+b] holds cluster q*128+p
        # (one fully contiguous DMA; host untangles)
        out = nc.dram_tensor("out", [128, NQ * B], mybir.dt.float32,
                             kind="ExternalOutput")
    else:
        # output c-major: [512 clusters, 256 batch] — each per-psum-tile
        # DMA writes a fully contiguous 128KB DRAM block
        out = nc.dram_tensor("out", [CLUSTERS_PER_CORE, B],
                             mybir.dt.float32, kind="ExternalOutput")

    xv, outv = x.ap(), out.ap()

    import contextlib
    with tile.TileContext(nc) as tc:
        with contextlib.ExitStack() as stk:
            xp = stk.enter_context(tc.tile_pool(name="xp", bufs=1))
            if quant == "int8":
                stp = stk.enter_context(tc.tile_pool(name="sp", bufs=1))
            ohp = stk.enter_context(tc.tile_pool(name="ohp", bufs=2))
            ps = stk.enter_context(
                tc.tile_pool(name="ps", bufs=2, space="PSUM"))
            resp = stk.enter_context(tc.tile_pool(name="res", bufs=2))
            def body(_i=None):
                # int8: keep the ACT queue free for dequant ops — all small
                # DMAs go via SWDGE (gpsimd), all x streams via sync
                small_eng = nc.gpsimd if quant == "int8" else nc.scalar
                if use_aux:
                    auxt = ohp.tile([128, npat + G + 2 * NQ], bf16,
                                    name="auxt", tag="auxt")
                    nc.scalar.dma_start(auxt[:], aux.ap())
                    cidt = auxt[:, :npat]
                    iot = auxt[:, npat:npat + G]
                    # recip rides along as fp32 bit-split into bf16 pairs
                    rect = auxt[:, npat + G:npat + G + 2 * NQ].bitcast(
                        mybir.dt.float32)
                else:
                    rect = ohp.tile([128, NQ], mybir.dt.float32,
                                    name="rect", tag="rect")
                    small_eng.dma_start(rect[:], recip.ap())
                if quant == "int8":
                    sgt = ohp.tile([128, GROUPS_PER_CORE],
                                   mybir.dt.float32, name="sgt", tag="sgt")
                    small_eng.dma_start(sgt[:], sg.ap())
                ohx = ohp.tile([128, npat, G], bf16, name="ohx",
                               tag="ohx")
                if SHIP_OH:
                    ohv = oh.ap()
                    for d in range(ndma):
                        s = slice(d * GPD * cpp, (d + 1) * GPD * cpp)
                        eng = nc.scalar if d % 2 == 0 else nc.sync
                        eng.dma_start(ohx[:, s, :], ohv[d])
                else:
                    if not use_aux:
                        cidt = ohp.tile([128, npat], bf16, name="cidt",
                                        tag="cidt")
                        small_eng.dma_start(cidt[:], cid.ap())
                        iot = ohp.tile([128, G], bf16, name="iot",
                                       tag="iot")
                        small_eng.dma_start(iot[:], iota.ap())
                    for g in range(GROUPS_PER_CORE):
                        s = slice(g * cpp, (g + 1) * cpp)
                        nc.vector.tensor_tensor(
                            out=ohx[:, s, :],
                            in0=cidt[:, s].unsqueeze(2)
                                .broadcast_to([128, cpp, G]),
                            in1=iot[:].unsqueeze(1)
                                .broadcast_to([128, cpp, G]),
                            op=mybir.AluOpType.is_equal,
                        )
                psum = [
                    ps.tile([128, kc * B], mybir.dt.float32,
                            name=f"psum{q}", tag=f"psum{q}")
                    for q in range(NQ)
                ]
                for d in range(ndma):
                    # alternate the two HWDGE rings (SP / ACT) so the
                    # per-dma fixed costs and streams run in parallel;
                    # int8 keeps ACT's queue clear -> all x on sync
                    eng = nc.sync if (quant == "int8" or d % 2 == 0) \
                        else nc.scalar
                    xt = xp.tile([128, GPD * cpg * B], xdt, tag=f"x{d}")
                    eng.dma_start(xt[:], xv[d])
                    if quant == "int8":
                        # dequantize each group slice int8 -> bf16 on
                        # DVE/ACT, scaling by the per-(group, partition)
                        # scalar during the copy
                        stag = []
                        for gg in range(GPD):
                            u = d * GPD + gg
                            st = stp.tile([128, cpg * B], bf16,
                                          tag=f"s{u % 12}")
                            sl = xt[:, gg * cpg * B:(gg + 1) * cpg * B]
                            sc = sgt[:, u:u + 1]
                            # DVE also builds the one-hot, so it gets 6 of
                            # the 16 dequant slices and ACT the other 10
                            if u % 3 == 0:
                                nc.vector.tensor_scalar_mul(st[:], sl, sc)
                            else:
                                nc.scalar.mul(st[:], sl, sc)
                            stag.append(st)
                    # issue the 4 groups of this block column-interleaved:
                    # consecutive matmuls hit different 32-col strips of the
                    # PE array (tile_position), so they run concurrently.
                    for t in range(cpp):
                        for gg in range(GPD):
                            g = d * GPD + gg
                            q, gq = divmod(g, gpq)
                            po = gq * G    # partition offset in psum tile
                            j = g * cpp + t
                            if quant == "int8":
                                rhs = stag[gg][:, t * B:(t + 1) * B]
                            else:
                                c0 = (gg * cpg + kc * t) * B
                                rhs = xt[:, c0:c0 + kc * B]
                            nc.tensor.matmul(
                                out=psum[q][po:po + G, :],
                                lhsT=ohx[:, j, :],
                                rhs=rhs,
                                start=(t == 0),
                                stop=(t == cpp - 1),
                                tile_position=(0, po),
                            )
                if use_aux:
                    # one wide result tile, scaled per psum tile, shipped
                    # with a single contiguous DMA
                    res_all = resp.tile([128, NQ * B], mybir.dt.float32,
                                        name="res_all", tag="res")
                    for q in range(NQ):
                        nc.vector.tensor_scalar_mul(
                            res_all[:, q * B:(q + 1) * B], psum[q][:],
                            rect[:, q:q + 1])
                    nc.sync.dma_start(outv[:], res_all[:])
                    return
                for q in range(NQ):
                    res = resp.tile([128, B], mybir.dt.float32,
                                    name=f"res{q}", tag="res")
                    if paired:
                        # the two pair-halves land side by side in PSUM;
                        # only one PSUM operand is allowed per op, so scale
                        # each half separately (DVE + ACT) and add in SBUF
                        tmp = resp.tile([128, B], mybir.dt.float32,
                                        name=f"tmp{q}", tag="tmp")
                        nc.vector.tensor_scalar_mul(
                            res[:], psum[q][:, :B], rect[:, q:q + 1])
                        nc.scalar.mul(tmp[:], psum[q][:, B:2 * B],
                                      rect[:, q:q + 1])
                        nc.vector.tensor_tensor(
                            out=res[:], in0=res[:], in1=tmp[:],
                            op=mybir.AluOpType.add,
                        )
                    else:
                        nc.vector.tensor_tensor(
                            out=res[:], in0=psum[q][:],
                            in1=rect[:, q:q + 1].broadcast_to([128, B]),
                            op=mybir.AluOpType.mult,
                        )
                    if quant == "int8":
                        nc.gpsimd.dma_start(
                            outv[q * 128:(q + 1) * 128, :], res[:])
                    else:
                        eng = nc.sync if q % 2 == 0 else nc.scalar
                        eng.dma_start(outv[q * 128:(q + 1) * 128, :],
                                      res[:])

            if loop == 1:
                body()
            else:
                with tc.For_i(0, loop, 1, staggered_reset=True) as i:
                    body(i)

    nc.compile()
    _program_cache[key] = nc
    return nc


def _solve_bins(counts: np.ndarray):
    """Partition the 4096 clusters into 128 bins of exactly 32 clusters,
    equalizing bin row-sums (ideally all == 2048 -> zero padding). Returns
    (bin_of, slot_of) int arrays."""
    n_bins = N_CLUSTERS // G
    target = int(counts.sum()) // n_bins
    rng = np.random.default_rng(0)
    orderd = np.argsort(-counts)
    bins = [[] for _ in range(n_bins)]
    sums = np.zeros(n_bins, dtype=np.int64)
    nitems = np.zeros(n_bins, dtype=np.int64)
    for c in orderd:
        cand = np.where(nitems < G)[0]
        b = int(cand[np.argmin(sums[cand])])
        bins[b].append(int(c))
        sums[b] += counts[c]
        nitems[b] += 1
    for _ in range(300000):
        dev = sums - target
        over = np.where(dev > 0)[0]
        under = np.where(dev < 0)[0]
        if len(over) == 0 or len(under) == 0:
            break
        A = int(rng.choice(over))
        Bb = int(rng.choice(under))
        ca, cb = bins[A], bins[Bb]
        diff = counts[ca][:, None] - counts[cb][None, :]
        tot = np.abs(dev[A] - diff) + np.abs(dev[Bb] + diff)
        i, j = np.unravel_index(int(np.argmin(tot)), tot.shape)
        if tot[i, j] < abs(dev[A]) + abs(dev[Bb]):
            a, b2 = ca[i], cb[j]
            ca.remove(a), cb.remove(b2)
            ca.append(b2), cb.append(a)
            d = counts[a] - counts[b2]
            sums[A] -= d
            sums[Bb] += d
    bin_of = np.zeros(N_CLUSTERS, dtype=np.int64)
    slot_of = np.zeros(N_CLUSTERS, dtype=np.int64)
    for b, cl in enumerate(bins):
        bin_of[cl] = b
        slot_of[cl] = np.arange(len(cl))
    return bin_of, slot_of, int(sums.max())


def _prepare(output: np.ndarray, mapping: np.ndarray):
    """Host prep: returns (nc, in_maps, cpg, unperm)."""
    t0 = time.time()
    assert output.shape == (32, 8, 512, 512) and output.dtype == np.float32
    mapping = np.asarray(mapping).astype(np.int64).ravel()
    assert mapping.shape == (N,)

    data2d = output.reshape(B, N)
    counts = np.bincount(mapping, minlength=N_CLUSTERS).astype(np.int64)
    recip = (1.0 / np.maximum(counts, 1)).astype(np.float32)

    order = np.argsort(mapping, kind="stable")
    cum = np.zeros(N_CLUSTERS + 1, dtype=np.int64)
    np.cumsum(counts, out=cum[1:])

    n_groups = N_CLUSTERS // G
    paired = PAIRED and QUANT == "bf16"
    # In paired mode a cluster's rows occupy whole chunk-pairs, so its
    # effective row count is rounded up to even.
    counts_eff = (counts + 1) // 2 * 2 if paired else counts
    # Bin-pack clusters into groups to minimize padding; fall back to
    # consecutive grouping if the packer leaves an oversized bin.
    bin_of, slot_of, maxsum = _solve_bins(counts_eff)
    naive_max = int(np.add.reduceat(counts_eff,
                                    np.arange(0, N_CLUSTERS, G)).max())
    if maxsum > naive_max:
        bin_of = np.arange(N_CLUSTERS) // G
        slot_of = np.arange(N_CLUSTERS) % G
        maxsum = naive_max
    if paired:
        cpg = max(2, 2 * int(np.ceil(maxsum / 256)))
    else:
        cpg = max(1, int(np.ceil(maxsum / 128)))
    L = 128 * cpg

    # clusters in destination order (bin-major, slot order)
    dest_order = np.lexsort((slot_of, bin_of))
    glen = np.zeros(n_groups, dtype=np.int64)
    np.add.at(glen, bin_of, counts_eff)
    rows_sorted = np.concatenate(
        [order[cum[c]:cum[c + 1]] for c in dest_order])
    gstart = np.zeros(n_groups + 1, dtype=np.int64)
    np.cumsum(glen, out=gstart[1:])

    if paired:
        # Pair-based placement: pair j of a cluster occupies positions
        # (chunk 2T, p) and (chunk 2T+1, p) of its group, so both chunks
        # of a pair share one partition->slot one-hot pattern.
        counts_dest = counts[dest_order]               # true counts
        pairs_of = (counts_dest + 1) // 2
        seg = np.zeros(len(dest_order) + 1, dtype=np.int64)
        np.cumsum(counts_dest, out=seg[1:])            # rows_sorted segs
        tot_pairs = int(pairs_of.sum())
        pstart = np.zeros(len(dest_order) + 1, dtype=np.int64)
        np.cumsum(pairs_of, out=pstart[1:])
        within = np.arange(tot_pairs) - np.repeat(pstart[:-1], pairs_of)
        r0 = np.repeat(seg[:-1], pairs_of) + 2 * within
        cnt_rep = np.repeat(counts_dest, pairs_of)
        has_r1 = (2 * within + 1) < cnt_rep
        # group and per-group pair index (dest order is bin-major)
        dpos = np.repeat(np.arange(len(dest_order)), pairs_of)
        pair_group = dpos // G
        pair_slot = (dpos % G).astype(np.int64)
        gp_pairs = np.add.reduceat(pairs_of, np.arange(0, N_CLUSTERS, G))
        gp_start = np.zeros(n_groups + 1, dtype=np.int64)
        np.cumsum(gp_pairs, out=gp_start[1:])
        ppi = np.arange(tot_pairs) - np.repeat(gp_start[:-1], gp_pairs)
        T = ppi // 128
        p = ppi % 128
        assert int(T.max()) < cpg // 2
        pad_rows = np.full(n_groups * L, -1, dtype=np.int64)
        base = pair_group * L + p
        pad_rows[base + (2 * T) * 128] = rows_sorted[r0]
        pad_rows[(base + (2 * T + 1) * 128)[has_r1]] = \
            rows_sorted[(r0 + 1)[has_r1]]
        vmask = pad_rows >= 0
        # per-pattern column id table [n_groups * (L//2)]
        cid_pat = np.zeros(n_groups * (L // 2), dtype=ml_dtypes.bfloat16)
        cid_pat[pair_group * (L // 2) + T * 128 + p] = \
            pair_slot.astype(ml_dtypes.bfloat16)
    else:
        # Padded row-id table [n_groups, L]; -1 = padding.
        pad_rows = np.full((n_groups, L), -1, dtype=np.int64)
        col = np.arange(L)
        valid = col[None, :] < glen[:, None]
        flat_src = np.zeros((n_groups, L), dtype=np.int64)
        flat_src[valid] = rows_sorted[
            (gstart[:-1][:, None]
             + np.minimum(col[None, :], glen[:, None] - 1))[valid]
        ]
        pad_rows[valid] = flat_src[valid]
        pad_rows = pad_rows.reshape(-1)        # [n_groups * L]
        vmask = pad_rows >= 0

    # Gather data rows (transposed): x_rows[r] = data2d[:, pad_rows[r]]
    dataT = np.ascontiguousarray(data2d.T)          # [N, B]
    if QUANT == "int8":
        # Magnitude-sorted placement: within each group, rank rows by
        # max|row| and place rank r at (chunk r%cpg, partition r//cpg) so
        # each partition holds rows of similar magnitude. Quantize with a
        # per-(group, partition) scale; the device applies it as a per-
        # partition scalar during the int8->bf16 dequant copy.
        pr2 = pad_rows.reshape(n_groups, L)
        vm2 = vmask.reshape(n_groups, L)
        rmax = np.full((n_groups, L), -1.0, dtype=np.float32)
        rmax[vm2] = np.abs(dataT[pr2[vm2]]).max(axis=1)
        rk = np.argsort(rmax, axis=1, kind="stable")   # padding first
        rows_rk = np.take_along_axis(pr2, rk, axis=1)  # rank-ordered rows
        rmax_rk = np.take_along_axis(rmax, rk, axis=1)
        # scale per (group, partition): partition p holds ranks
        # [p*cpg, (p+1)*cpg)
        sgmat = (np.maximum(rmax_rk.reshape(n_groups, 128, cpg).max(axis=2),
                            1e-30) / 127.0).astype(np.float32)
        # rank r -> position (chunk r%cpg)*128 + (partition r//cpg)
        pos = (np.arange(L) % cpg) * 128 + (np.arange(L) // cpg)
        pr_new = np.empty_like(pr2)
        np.put_along_axis(pr_new, np.broadcast_to(pos, (n_groups, L)),
                          rows_rk, axis=1)
        pad_rows = pr_new.reshape(-1)
        vmask = pad_rows >= 0
        # per-position scale = scale of its partition
        s_pos = sgmat[:, np.arange(L) % 128].reshape(-1)   # [n_groups*L]
        x_rows = np.zeros((n_groups * L, B), dtype=np.int8)
        x_rows[vmask] = np.clip(
            np.round(dataT[pad_rows[vmask]] / s_pos[vmask][:, None]),
            -127, 127).astype(np.int8)
        # pack scales per core: [NCORES, 128, GROUPS_PER_CORE]
        sg_all = np.ascontiguousarray(
            sgmat.reshape(NCORES, GROUPS_PER_CORE, 128).transpose(0, 2, 1))
    else:
        x_rows = np.zeros((n_groups * L, B), dtype=ml_dtypes.bfloat16)
        x_rows[vmask] = dataT[pad_rows[vmask]].astype(ml_dtypes.bfloat16)
    # pack partition-major per DMA block: [dma, t, p, b] -> [dma, p, t*B+b]
    n_dma = n_groups // GPD
    x_all = np.ascontiguousarray(
        x_rows.reshape(n_dma, GPD * cpg, 128, B).transpose(0, 2, 1, 3)
    ).reshape(n_dma, 128, GPD * cpg * B)

    # Compact one-hot: per-row within-group column id (bf16).
    if paired:
        cid_all = cid_pat          # one column id per chunk-pair pattern
    else:
        cid_all = np.zeros(n_groups * L, dtype=ml_dtypes.bfloat16)
        clus = mapping[pad_rows[vmask]]
        cid_all[vmask] = slot_of[clus].astype(ml_dtypes.bfloat16)
    if SHIP_OH:
        # host-prebuilt one-hot [rows, G]
        clus = mapping[pad_rows[vmask]]
        w_rows = np.zeros(n_groups * L, dtype=np.float32)
        w_rows[vmask] = 1.0
        slot_rows = np.zeros(n_groups * L, dtype=np.int16)
        slot_rows[vmask] = slot_of[clus]
        oh_rows = (slot_rows[:, None] == np.arange(G, dtype=np.int16)[None]
                   ).astype(np.float32) * w_rows[:, None]
        # pack like x: [dma, chunk, p, G] -> [dma, p, chunk*G]
        oh_all = np.ascontiguousarray(
            oh_rows.reshape(n_dma, GPD * cpg, 128, G).transpose(0, 2, 1, 3)
        ).reshape(n_dma, 128, GPD * cpg * G).astype(ml_dtypes.bfloat16)
    # where cluster c ended up in the concatenated [4096, B] device output
    unperm = bin_of * G + slot_of
    # per-core per-psum-tile per-partition reciprocal counts
    counts_dest = counts[dest_order]               # [4096] device order
    recip_dev = (1.0 / np.maximum(counts_dest, 1)).astype(np.float32)
    recip_all = recip_dev.reshape(NCORES, NQ, 128).transpose(0, 2, 1)
    recip_all = np.ascontiguousarray(recip_all)    # [NCORES, 128, NQ]
    # pack [rows] -> [core][p][chunk]
    npat_core = GROUPS_PER_CORE * (cpg // 2 if paired else cpg)

    def pack(a):
        return np.ascontiguousarray(
            a.reshape(NCORES, npat_core, 128).transpose(0, 2, 1))

    cid_all = pack(cid_all)
    iota_np = np.broadcast_to(
        np.arange(G, dtype=ml_dtypes.bfloat16), (128, G)).copy()

    t1 = time.time()
    nc = _build_program(cpg)

    ndma_core = GROUPS_PER_CORE // GPD
    use_aux = FUSE_AUX and QUANT == "bf16" and not SHIP_OH and not paired
    in_maps = []
    for k in range(NCORES):
        m = {"x": x_all[k * ndma_core:(k + 1) * ndma_core]}
        if use_aux:
            m["aux"] = np.ascontiguousarray(np.concatenate([
                np.asarray(cid_all[k]),
                iota_np,
                np.ascontiguousarray(recip_all[k])
                .view(ml_dtypes.bfloat16),
            ], axis=1))
        else:
            m["recip"] = recip_all[k]
            if SHIP_OH:
                m["oh"] = oh_all[k * ndma_core:(k + 1) * ndma_core]
            else:
                m["cid"] = cid_all[k]
                m["iota"] = iota_np
        if QUANT == "int8":
            m["sg"] = sg_all[k]
        in_maps.append(m)
    print(f"[kernel] host prep {t1 - t0:.2f}s  build+compile "
          f"{time.time() - t1:.2f}s  (cpg={cpg}, quant={QUANT})",
          file=sys.stderr, flush=True)
    return nc, in_maps, cpg, unperm


def kernel(output: np.ndarray, mapping: np.ndarray) -> np.ndarray:
    nc, in_maps, _, unperm = _prepare(output, mapping)
    t2 = time.time()
    res = run_bass_kernel_spmd(nc, in_maps, list(range(NCORES)))
    t3 = time.time()
    if FUSE_AUX and QUANT == "bf16" and not PAIRED and not SHIP_OH:
        # device out is partition-major [128, NQ*B] -> untangle
        full = np.concatenate([
            res.results[k]["out"].reshape(128, NQ, B)
            .transpose(1, 0, 2).reshape(CLUSTERS_PER_CORE, B)
            for k in range(NCORES)
        ], axis=0)                                  # [4096, 256] device order
    else:
        full = np.concatenate([res.results[k]["out"]
                               for k in range(NCORES)], axis=0)
    full = full[unperm]                             # -> cluster order
    out = np.ascontiguousarray(full.T).reshape(32, 8, N_CLUSTERS)
    print(f"[kernel] run {t3 - t2:.2f}s", file=sys.stderr, flush=True)
    return out
